# revision 1
# baseline (speedup 1.0000x reference)
"""Trainium2 Bass kernel: 4096x4096 fp32 image, 7x7 valid cross-correlation + bias.

Strategy
--------
Column-shard the image across 8 NeuronCores: core m computes output columns
[512*m, 512*m+512) (core 7 padded; image columns are padded to 4102 on host so
every core sees an identical [4154+, 518] fp16 input shard = 512 columns + 6
halo columns, all 4096 rows + zero-padded tail rows).

On each core the conv runs on the tensor engine as banded-Toeplitz matmuls:
for an output row band of M=122 rows we load K=128 input rows (M + kh-1) as the
moving operand [128, 518] and contract against seven stationary matrices
A_dj[128, 122], A_dj[k, m] = w[k-m, dj].  The seven column taps dj become
free-axis shifts of the moving operand (rhs = x[:, dj:dj+512]) accumulated in
one PSUM bank via start/stop.  Eviction PSUM->SBUF fuses the scalar bias add on
the scalar engine; DMAs are batched 8 bands at a time (~1-2 MB per transfer).

Inputs are cast to fp16 on host (PE runs 16-bit ops at full rate, PSUM
accumulates fp32; fp16 keeps 11 mantissa bits -> ~5e-4 worst-case rel err).
"""

import os
import sys

import numpy as np

for _p in ("/root/.axon_site/_ro/trn_rl_repo", "/opt/trn_rl_repo"):
    if os.path.isdir(_p) and _p not in sys.path:
        sys.path.append(_p)

H = W = 4096
KH = KW = 7
OH = OW = H - KH + 1            # 4090
NCORES = 8
CW = 512                        # output columns per core
CIN = CW + KW - 1               # 518 input columns per core (incl. halo)
BAND = 128 - (KH - 1)           # 122 output rows per band
NBANDS = -(-OH // BAND)         # 34
ROWS_PAD = BAND * (NBANDS - 1) + 128    # 4154 input rows incl. zero tail
OUT_PAD = BAND * NBANDS         # 4148 output rows incl. junk tail
GROUP = 8                       # bands per DMA batch

_prog = None


def _program():
    global _prog
    if _prog is not None:
        return _prog

    from contextlib import ExitStack

    import concourse.bass as bass
    import concourse.tile as tile
    from concourse import bacc, mybir

    nc = bacc.Bacc("TRN2", target_bir_lowering=False, debug=False)
    xs = nc.dram_tensor("xs", [ROWS_PAD, CIN], mybir.dt.float16, kind="ExternalInput")
    ab = nc.dram_tensor("ab", [KW, 128, BAND], mybir.dt.float16, kind="ExternalInput")
    br = nc.dram_tensor("br", [128, 1], mybir.dt.float32, kind="ExternalInput")
    yd = nc.dram_tensor("yd", [OUT_PAD, CW], mybir.dt.float32, kind="ExternalOutput")

    with tile.TileContext(nc) as tc, ExitStack() as ctx:
        consts = ctx.enter_context(tc.tile_pool(name="consts", bufs=1))
        inp = ctx.enter_context(tc.tile_pool(name="inp", bufs=3))
        pss = ctx.enter_context(tc.tile_pool(name="pss", bufs=4, space="PSUM"))
        outp = ctx.enter_context(tc.tile_pool(name="outp", bufs=2))

        a_t = consts.tile([128, KW, BAND], mybir.dt.float16)
        nc.sync.dma_start(
            a_t[:, :, :],
            bass.AP(ab, 0, [[BAND, 128], [128 * BAND, KW], [1, BAND]]),
        )
        b_t = consts.tile([128, 1], mybir.dt.float32)
        nc.sync.dma_start(b_t[:, :], br.ap())

        b0 = 0
        while b0 < NBANDS:
            g = min(GROUP, NBANDS - b0)
            # [row-in-band, band, col]; bands overlap by kh-1 rows in DRAM
            xin = inp.tile([128, GROUP, CIN], mybir.dt.float16, tag="xin")
            nc.sync.dma_start(
                xin[:, :g, :],
                bass.AP(
                    xs,
                    b0 * BAND * CIN,
                    [[CIN, 128], [BAND * CIN, g], [1, CIN]],
                ),
            )
            yo = outp.tile([128, GROUP, CW], mybir.dt.float32, tag="yo")
            for i in range(g):
                ps = pss.tile([128, CW], mybir.dt.float32, tag="ps")
                for dj in range(KW):
                    nc.tensor.matmul(
                        ps[0:BAND, :],
                        a_t[:, dj, :],
                        xin[:, i, dj : dj + CW],
                        start=(dj == 0),
                        stop=(dj == KW - 1),
                    )
                nc.scalar.activation(
                    yo[0:BAND, i, :],
                    ps[0:BAND, :],
                    mybir.ActivationFunctionType.Identity,
                    bias=b_t[0:BAND, :],
                    scale=1.0,
                )
            nc.sync.dma_start(
                bass.AP(
                    yd,
                    b0 * BAND * CW,
                    [[CW, BAND], [BAND * CW, g], [1, CW]],
                ),
                yo[0:BAND, :g, :],
            )
            b0 += g

    nc.compile()
    _prog = nc
    return nc


def _shards(x, weight, bias):
    x = np.asarray(x, dtype=np.float32)
    weight = np.asarray(weight, dtype=np.float32)
    bias = np.asarray(bias, dtype=np.float32)

    xp = np.zeros((ROWS_PAD, NCORES * CW + (KW - 1)), dtype=np.float16)
    xp[:H, :W] = x.astype(np.float16)

    wh = weight.astype(np.float16)
    abm = np.zeros((KW, 128, BAND), dtype=np.float16)
    idx = np.arange(BAND)
    for dj in range(KW):
        for di in range(KH):
            abm[dj, idx + di, idx] = wh[di, dj]

    brep = np.full((128, 1), np.float32(bias[0]), dtype=np.float32)

    return [
        {
            "xs": np.ascontiguousarray(xp[:, m * CW : m * CW + CIN]),
            "ab": abm,
            "br": brep,
        }
        for m in range(NCORES)
    ]


def _gather(results):
    y = np.empty((OH, OW), dtype=np.float32)
    for m in range(NCORES):
        c0 = m * CW
        c1 = min(c0 + CW, OW)
        y[:, c0:c1] = results[m]["yd"][:OH, : c1 - c0]
    return y


def kernel(x, weight, bias):
    from concourse.bass_utils import run_bass_kernel_spmd

    nc = _program()
    in_maps = _shards(x, weight, bias)
    res = run_bass_kernel_spmd(nc, in_maps, core_ids=list(range(NCORES)))
    return _gather(res.results)


# revision 3
# speedup vs baseline: 1.4863x; 1.4863x over previous
"""Trainium2 Bass kernel: 4096x4096 fp32 image, 7x7 valid cross-correlation + bias.

Strategy
--------
Column-shard the image across 8 NeuronCores: core m computes output columns
[512*m, 512*m+512) (core 7 padded; image columns padded to 4102 on host, so
every core sees an identical input shard = 512 columns + 6 halo columns).

On each core the conv runs on the tensor engine as banded-Toeplitz matmuls:
an output row band of M=122 rows uses K=128 input rows (M + kh-1) as the
moving operand and contracts against seven stationary matrices A_dj[128, 128],
A_dj[k, m] = w[k-m, dj] (zero outside the band / beyond column 121).  The
seven column taps dj become free-axis shifts of the moving operand
(rhs = x[:, dj:dj+512]) accumulated in one PSUM bank via start/stop.

Layout: the host prepacks each shard band-partition-major, xs[p, b, c] =
x[122*b + p, c], so one DMA of G=8 bands reads a contiguous 8.3 KB run per
partition (128 descriptors/transfer instead of 1024).  Same for the output.
Matmuls run dj-major across the 8 PSUM banks of a group so each stationary is
loaded once per group.  Loads issue on the Sync HWDGE ring, stores on the
GpSimd SWDGE ring, PSUM eviction (+ fused scalar bias add) on the scalar
engine.  Inputs are cast to fp16 on host (PE runs 16-bit at full rate, PSUM
accumulates fp32; fp16 keeps 11 mantissa bits -> ~3e-4 rel err).
"""

import os
import sys

import numpy as np

for _p in ("/root/.axon_site/_ro/trn_rl_repo", "/opt/trn_rl_repo"):
    if os.path.isdir(_p) and _p not in sys.path:
        sys.path.append(_p)

H = W = 4096
KH = KW = 7
OH = OW = H - KH + 1            # 4090
NCORES = 8
CW = 512                        # output columns per core
CIN = CW + KW - 1               # 518 input columns per core (incl. halo)
BAND = 128 - (KH - 1)           # 122 output rows per band
NBANDS = -(-OH // BAND)         # 34
ROWS_PAD = BAND * (NBANDS - 1) + 128    # 4154 input rows incl. zero tail
OUT_PAD = BAND * NBANDS         # 4148 output rows incl. junk tail
GROUP = 8                       # bands per DMA batch / PSUM-bank rotation

_prog = None


def _program():
    global _prog
    if _prog is not None:
        return _prog

    from contextlib import ExitStack

    import concourse.bass as bass
    import concourse.tile as tile
    from concourse import bacc, mybir

    nc = bacc.Bacc("TRN2", target_bir_lowering=False, debug=False)
    xs = nc.dram_tensor(
        "xs", [128, NBANDS, CIN], mybir.dt.float16, kind="ExternalInput"
    )
    ab = nc.dram_tensor("ab", [KW, 128, 128], mybir.dt.float16, kind="ExternalInput")
    br = nc.dram_tensor("br", [128, 1], mybir.dt.float32, kind="ExternalInput")
    yd = nc.dram_tensor(
        "yd", [BAND, NBANDS, CW], mybir.dt.float32, kind="ExternalOutput"
    )
    xs_ap, ab_ap, br_ap, yd_ap = xs.ap(), ab.ap(), br.ap(), yd.ap()

    with tile.TileContext(nc) as tc, ExitStack() as ctx:
        consts = ctx.enter_context(tc.tile_pool(name="consts", bufs=1))
        inp = ctx.enter_context(tc.tile_pool(name="inp", bufs=3))
        pss = ctx.enter_context(tc.tile_pool(name="pss", bufs=GROUP, space="PSUM"))
        outp = ctx.enter_context(tc.tile_pool(name="outp", bufs=2))

        a_t = consts.tile([128, KW, 128], mybir.dt.float16)
        nc.sync.dma_start(
            a_t[:, :, :],
            bass.AP(ab, 0, [[128, 128], [128 * 128, KW], [1, 128]]),
        )
        b_t = consts.tile([128, 1], mybir.dt.float32)
        nc.sync.dma_start(b_t[:, :], br_ap)

        b0 = 0
        while b0 < NBANDS:
            g = min(GROUP, NBANDS - b0)
            xin = inp.tile([128, GROUP, CIN], mybir.dt.float16, tag="xin")
            nc.sync.dma_start(xin[:, :g, :], xs_ap[:, b0 : b0 + g, :])

            yo = outp.tile([128, GROUP, CW], mybir.dt.float32, tag="yo")
            pst = [
                pss.tile([128, CW], mybir.dt.float32, tag="ps", name=f"ps{b0}_{i}")
                for i in range(g)
            ]
            for dj in range(KW):
                for i in range(g):
                    nc.tensor.matmul(
                        pst[i][:, :],
                        a_t[:, dj, :],
                        xin[:, i, dj : dj + CW],
                        start=(dj == 0),
                        stop=(dj == KW - 1),
                    )
            for i in range(g):
                nc.scalar.activation(
                    yo[0:BAND, i, :],
                    pst[i][0:BAND, :],
                    mybir.ActivationFunctionType.Identity,
                    bias=b_t[0:BAND, :],
                    scale=1.0,
                )
            nc.gpsimd.dma_start(yd_ap[:, b0 : b0 + g, :], yo[0:BAND, :g, :])
            b0 += g

    nc.compile()
    _prog = nc
    return nc


def _shards(x, weight, bias):
    x = np.asarray(x, dtype=np.float32)
    weight = np.asarray(weight, dtype=np.float32)
    bias = np.asarray(bias, dtype=np.float32)

    xp = np.zeros((ROWS_PAD, NCORES * CW + (KW - 1)), dtype=np.float16)
    xp[:H, :W] = x.astype(np.float16)

    wh = weight.astype(np.float16)
    abm = np.zeros((KW, 128, 128), dtype=np.float16)
    idx = np.arange(BAND)
    for dj in range(KW):
        for di in range(KH):
            abm[dj, idx + di, idx] = wh[di, dj]

    brep = np.full((128, 1), np.float32(bias[0]), dtype=np.float32)

    s0, s1 = xp.strides
    ins = []
    for m in range(NCORES):
        core = xp[:, m * CW : m * CW + CIN]
        # xs[p, b, c] = core[BAND*b + p, c] -- overlapping-band strided view
        xb = np.lib.stride_tricks.as_strided(
            core, shape=(128, NBANDS, CIN), strides=(s0, BAND * s0, s1)
        )
        ins.append({"xs": np.ascontiguousarray(xb), "ab": abm, "br": brep})
    return ins


def _gather(results):
    y = np.empty((OH, OW), dtype=np.float32)
    for m in range(NCORES):
        c0 = m * CW
        c1 = min(c0 + CW, OW)
        # yd[r, b, c] = out[BAND*b + r, c]
        full = results[m]["yd"].transpose(1, 0, 2).reshape(OUT_PAD, CW)
        y[:, c0:c1] = full[:OH, : c1 - c0]
    return y


def kernel(x, weight, bias):
    from concourse.bass_utils import run_bass_kernel_spmd

    nc = _program()
    in_maps = _shards(x, weight, bias)
    res = run_bass_kernel_spmd(nc, in_maps, core_ids=list(range(NCORES)))
    return _gather(res.results)
